# revision 28
# baseline (speedup 1.0000x reference)
"""Multi-head causal self-attention (B=2, S=2048, H=2048, NH=16) on 8 TRN2
NeuronCores.

Sharding: data-parallel over batch (2 groups of 4 cores) x tensor-parallel
over heads (4 heads per core; q/k/v projections column-split, output
projection row-split). Each core computes a partial [S, H] output-projection
product; the host sums the 4 partials per batch and adds the output bias.

Per-core device kernel (all matmul inputs bf16, fp32 PSUM accumulation):
  phase 1: QT/KT per head [128d, S] (1/sqrt(hd) folded into Wq on host),
           V as [S, 512] natural layout
  phase 2: per (head, q-chunk of 512): causal scores in [k, q] orientation,
           exp on ACT, triangle-mask multiply on diagonal k-tiles,
           denominator via ones-matmul, PV accumulated as outT [d, q],
           normalize via reciprocal + partition-broadcast
  phase 3: partial output projection [S, H] -> DRAM fp32

All per-core data rides in two DRAM inputs (wall: ht|weights|mask as bf16
rows, bias: f32) to keep the per-dispatch argument-marshaling cost low on
the runtime side. Phase 2 runs as a flat (head, q-chunk, k-tile) stream
with a 4-deep scores->exp software pipeline crossing chunk boundaries.
"""

import math
import sys

if "/opt/trn_rl_repo" not in sys.path:
    sys.path.insert(0, "/opt/trn_rl_repo")

import numpy as np
import ml_dtypes

import concourse.bass as bass
import concourse.mybir as mybir
import concourse.tile as tile
from concourse.bass_utils import run_bass_kernel_spmd

B, S, H, NH = 2, 2048, 2048, 16
HD = H // NH            # 128
NCORES = 8
HPC = NH // 4           # 4 heads per core
DSH = HPC * HD          # 512 per-core head-dim shard
P = 128                 # partitions
NT = S // P             # 16 s/k tiles of 128
NJ = S // 512           # 4 q/s chunks of 512
BF16 = mybir.dt.bfloat16
F32 = mybir.dt.float32

# wall row ranges (each row is 512 bf16); ht rides in front as (H*4, 512)
_HT0 = 0
_WQ0, _WK0, _WV0, _WO0, _MSK0 = 8192, 10240, 12288, 14336, 16384
_WALL_ROWS = 16896

_NEG_BIG = -1.0e8  # masked entries in the reference mask are <= -1e9


def _split_excess_waits(nc, max_waits: int = 1) -> int:
    """This container's walrus rejects >1 sync wait per instruction
    ("Too many sync wait commands" in setupSyncWait). Hoist excess waits
    onto preceding same-engine NoOps; waits still execute in engine order
    before the original instruction, so sync semantics are unchanged."""
    n_split = 0
    for f in nc.m.functions:
        for bb in f.blocks:
            insts = bb.instructions
            out = []
            changed = False
            for inst in insts:
                si = inst.sync_info
                if si is not None and len(si.on_wait) > max_waits:
                    waits = list(si.on_wait)
                    excess, keep = waits[:-max_waits], waits[-max_waits:]
                    for i in range(0, len(excess), max_waits):
                        chunk = excess[i : i + max_waits]
                        nop = mybir.InstNoOp(
                            name=f"{inst.name}-waitsplit-{i}", ins=[], outs=[]
                        )
                        nop.engine = inst.engine
                        nop.sync_info = mybir.SyncInfo(on_wait=chunk, on_update=[])
                        nc.register_instruction(nop)
                        out.append(nop)
                    inst.sync_info = mybir.SyncInfo(
                        on_wait=keep, on_update=list(si.on_update)
                    )
                    changed = True
                    n_split += 1
                out.append(inst)
            if changed:
                bb.instructions = out
    return n_split


def _build_nc():
    nc = bass.Bass()
    # wall: ht | wqt | wkt | wvt | wot | msk stacked as (16896, 512) bf16
    # rows — a single input tensor keeps per-dispatch marshaling minimal
    wall = nc.dram_tensor("wall", (_WALL_ROWS, 512), BF16, kind="ExternalInput")
    # bias: [bq2 (4) | bk2 (4) | bvb (512)] as (128, 520) f32 columns
    bias = nc.dram_tensor("bias", (P, 520), F32, kind="ExternalInput")
    # partial products are summed on the host; bf16 partials halve the
    # output traffic and cost <0.1% relative error on the final sum
    o = nc.dram_tensor("o", (S, H), BF16, kind="ExternalOutput")

    with tile.TileContext(nc) as tc:
        with (
            tc.tile_pool(name="wpool", bufs=1) as wpool,
            tc.tile_pool(name="cpool", bufs=1) as cpool,
            tc.tile_pool(name="hpool", bufs=2) as hpool,
            tc.tile_pool(name="qkpool", bufs=1) as qkpool,
            tc.tile_pool(name="epool", bufs=6) as epool,
            tc.tile_pool(name="rpool", bufs=2) as rpool,
            tc.tile_pool(name="opool", bufs=4) as opool,
            tc.tile_pool(name="ps_mm", bufs=3, space="PSUM") as ps_mm,
            tc.tile_pool(name="ps_out", bufs=2, space="PSUM") as ps_out,
            tc.tile_pool(name="ps_den", bufs=2, space="PSUM") as ps_den,
            tc.tile_pool(name="ps_rb", bufs=1, space="PSUM") as ps_rb,
        ):
            # ---- constants / weights into SBUF ----
            # Load order matters: the first Q matmuls need wq + the first
            # hidden chunk; split the big loads in 4 so they spread across
            # DMA queues and compute starts as early as possible. wo is not
            # needed until phase 3 and is loaded right before it.
            wq_sb = wpool.tile([P, NT, DSH], BF16)
            wk_sb = wpool.tile([P, NT, DSH], BF16)
            wv_sb = wpool.tile([P, NT, DSH], BF16)
            wqt_r = wall[_WQ0 : _WQ0 + 2048].rearrange("(t p) m -> p t m", p=P)
            wkt_r = wall[_WK0 : _WK0 + 2048].rearrange("(t p) m -> p t m", p=P)
            wvt_r = wall[_WV0 : _WV0 + 2048].rearrange("(t p) m -> p t m", p=P)
            # finest split for the first-needed tiles, alternating the two
            # tensors the first accumulation reads: the j=0 Q pass can begin
            # as soon as hidden tile 0 + wq tile 0 arrive
            h0_sb = hpool.tile([P, NT, 512], BF16, tag="h")
            # ht element (h, s): row h*4 + s//512, col s%512 of the wall
            hall = wall[_HT0 : _HT0 + 8192].rearrange(
                "(t p a) w -> p t a w", p=P, a=4
            )
            ht_r0 = hall[:, :, 0, :]
            # single-tile starters so the first Q matmuls begin ASAP, then
            # progressively larger chunks: each DMA instruction costs ~0.6us
            # of serialized descriptor generation, so the bulk rides in few
            # instructions, with wk/wv interleaved early enough that the
            # K/V passes never wait
            for t in range(2):
                nc.sync.dma_start(h0_sb[:, t, :], ht_r0[:, t, :])
                nc.sync.dma_start(wq_sb[:, t, :], wqt_r[:, t, :])
            for t4 in (slice(2, 4), slice(4, 8), slice(8, 12), slice(12, 16)):
                nc.sync.dma_start(h0_sb[:, t4, :], ht_r0[:, t4, :])
                nc.sync.dma_start(wq_sb[:, t4, :], wqt_r[:, t4, :])
            bias_sb = cpool.tile([P, 520], F32)
            nc.sync.dma_start(bias_sb[:], bias[:, :])
            bq_sb = bias_sb[:, 0:4]
            bk_sb = bias_sb[:, 4:8]
            bv_sb = bias_sb[:, 8:520]
            mask_sb = cpool.tile([P, 4, 512], BF16)
            msk_r = wall[_MSK0 : _MSK0 + 512].rearrange("(p r) w -> p r w", p=P)
            nc.sync.dma_start(mask_sb[:], msk_r[:, :, :])
            ones_sb = cpool.tile([P, 1], BF16)
            nc.vector.memset(ones_sb[:], 1.0)
            onesrow_f32 = cpool.tile([1, P], F32)
            nc.vector.memset(onesrow_f32[:], 1.0)
            onesrow_sb = cpool.tile([1, P], mybir.dt.float32r)
            nc.vector.tensor_copy(onesrow_sb[:], onesrow_f32[:])

            qt_sb = qkpool.tile([P, HPC, S], BF16)   # per-head Q^T [d, s]
            kt_sb = qkpool.tile([P, HPC, S], BF16)   # per-head K^T [d, s]
            v_sb = qkpool.tile([P, NT, DSH], BF16)   # V [s-tile, d]
            ao_sb = qkpool.tile([P, HPC, S], BF16)   # attn-out^T [d, q] per head

            # ---- phase 1: QKV projections ----
            for j in range(NJ):
                sj = slice(512 * j, 512 * (j + 1))
                if j == 0:
                    h_sb = h0_sb
                    # K/V weights arrive while the j=0 Q pass computes
                    for t8 in (slice(0, 8), slice(8, 16)):
                        nc.sync.dma_start(wk_sb[:, t8, :], wkt_r[:, t8, :])
                    for t8 in (slice(0, 8), slice(8, 16)):
                        nc.sync.dma_start(wv_sb[:, t8, :], wvt_r[:, t8, :])
                else:
                    h_sb = hpool.tile([P, NT, 512], BF16, tag="h")
                    for t8 in (slice(0, 8), slice(8, 16)):
                        nc.sync.dma_start(h_sb[:, t8, :], hall[:, t8, j, :])
                for hd in range(HPC):
                    md = slice(HD * hd, HD * (hd + 1))
                    acc_q = ps_mm.tile([P, 512], F32, tag="mm")
                    for t in range(NT):
                        nc.tensor.matmul(
                            acc_q[:], wq_sb[:, t, md], h_sb[:, t, :],
                            start=(t == 0), stop=(t == NT - 1),
                        )
                    nc.vector.tensor_scalar_add(
                        qt_sb[:, hd, sj], acc_q[:], bq_sb[:, hd : hd + 1]
                    )
                for hd in range(HPC):
                    md = slice(HD * hd, HD * (hd + 1))
                    acc_k = ps_mm.tile([P, 512], F32, tag="mm")
                    for t in range(NT):
                        nc.tensor.matmul(
                            acc_k[:], wk_sb[:, t, md], h_sb[:, t, :],
                            start=(t == 0), stop=(t == NT - 1),
                        )
                    nc.vector.tensor_scalar_add(
                        kt_sb[:, hd, sj], acc_k[:], bk_sb[:, hd : hd + 1]
                    )
                for st in range(4):
                    ms = slice(P * st, P * (st + 1))
                    acc_v = ps_mm.tile([P, DSH], F32, tag="mm")
                    for t in range(NT):
                        nc.tensor.matmul(
                            acc_v[:], h_sb[:, t, ms], wv_sb[:, t, :],
                            start=(t == 0), stop=(t == NT - 1),
                        )
                    nc.vector.tensor_add(v_sb[:, 4 * j + st, :], acc_v[:], bv_sb[:])

            # ---- phase 2: causal attention, [k, q] orientation ----
            # wo arrives during phase 2; it is only read by outproj
            wo_sb = wpool.tile([P, HPC, H], BF16)
            wot_r = wall[_WO0 : _WO0 + 2048].rearrange(
                "(t p a) w -> p t (a w)", p=P, a=4
            )
            for q4 in range(4):
                nc.sync.dma_start(wo_sb[:, q4, :], wot_r[:, q4, :])
            def _normalize(pend):
                # divide the accumulated outT by the softmax denominator:
                # one f32r partition-reduce matmul over the DVE-accumulated
                # partial sums, reciprocal on DVE, partition-broadcast via a
                # PE ones-matmul, then a multiply into the bf16 attn-out tile
                ot_ps, den_ps, n_hd, n_sj = pend
                rc = rpool.tile([1, 512], F32, tag="rc")
                nc.vector.reciprocal(rc[:], den_ps[:])
                # float32r matmul is 4x faster than fp32 at N>=256; the BIR
                # verifier requires producers that round to f32r, hence the
                # explicit converting copies
                rc_r = rpool.tile([1, 512], mybir.dt.float32r, tag="rcr")
                nc.vector.tensor_copy(rc_r[:], rc[:])
                rb_ps = ps_rb.tile([P, 512], F32, tag="rb")
                nc.tensor.matmul(
                    rb_ps[:], onesrow_sb[:], rc_r[:], start=True, stop=True
                )
                rb = rpool.tile([P, 512], F32, tag="rb")
                nc.vector.tensor_copy(rb[:], rb_ps[:])
                nc.vector.tensor_mul(ao_sb[:, n_hd, n_sj], ot_ps[:], rb[:])

            def _issue_scores(hd, j, k):
                # scores matmul + exp (+ triangle mask on diagonal tiles);
                # diagonal k-tiles r>=1 narrow to the exact valid column
                # range (q < 128r is fully masked)
                kd = slice(P * k, P * (k + 1))
                r = k - 4 * j
                q0 = 128 * r if r >= 1 else 0
                qv = slice(q0, 512)
                st_ps = ps_mm.tile([P, 512], F32, tag="mm")
                nc.tensor.matmul(
                    st_ps[:, qv], kt_sb[:, hd, kd],
                    qt_sb[:, hd, 512 * j + q0 : 512 * (j + 1)],
                    start=True, stop=True,
                )
                e = epool.tile([P, 512], BF16, tag="e")
                nc.scalar.activation(
                    e[:, qv], st_ps[:, qv], mybir.ActivationFunctionType.Exp
                )
                if r >= 0:
                    nc.vector.tensor_mul(e[:, qv], e[:, qv], mask_sb[:, r, qv])
                return e, qv, r

            # flat (hd, j, k) step stream with a two-deep scores/exp
            # pipeline that crosses chunk boundaries, so den/PV never wait
            # on ACT's exp latency — not even on the first k of a chunk
            steps = [
                (hd, j, k)
                for hd in range(HPC)
                for j in range(NJ)
                for k in range(4 * j + 4)
            ]
            PIPE = 4
            fifo = [_issue_scores(*steps[i]) for i in range(PIPE)]
            pending = None
            ot_ps = den_ps = None
            for idx, (hd, j, k) in enumerate(steps):
                kmax = 4 * j + 4
                if k == 0:
                    ot_ps = ps_out.tile([P, 512], F32, tag="ot")
                    den_ps = ps_den.tile([1, 512], F32, tag="den")
                e, qv, r = fifo.pop(0)
                if idx + PIPE < len(steps):
                    fifo.append(_issue_scores(*steps[idx + PIPE]))
                nc.tensor.matmul(
                    den_ps[:, qv], ones_sb[:], e[:, qv],
                    start=(k == 0), stop=(k == kmax - 1),
                    skip_group_check=(r >= 1),
                )
                nc.tensor.matmul(
                    ot_ps[:, qv], v_sb[:, k, HD * hd : HD * (hd + 1)],
                    e[:, qv],
                    start=(k == 0), stop=(k == kmax - 1),
                    skip_group_check=(r >= 1),
                )
                if k == 1 and pending is not None:
                    # normalize the previous (head, chunk) one group late,
                    # so its PE matmuls never stall on the DVE
                    # accumulation / reciprocal latency
                    _normalize(pending)
                    pending = None
                if k == kmax - 1:
                    pending = (
                        ot_ps, den_ps, hd, slice(512 * j, 512 * (j + 1))
                    )
            _normalize(pending)

            # ---- phase 3: partial output projection ----
            for si in range(NT):
                rs = slice(P * si, P * (si + 1))
                for c in range(NJ):
                    hc = slice(512 * c, 512 * (c + 1))
                    acc_o = ps_mm.tile([P, 512], F32, tag="mm")
                    for dt in range(HPC):
                        nc.tensor.matmul(
                            acc_o[:], ao_sb[:, dt, rs], wo_sb[:, dt, hc],
                            start=(dt == 0), stop=(dt == HPC - 1),
                        )
                    oc = opool.tile([P, 512], BF16, tag="oc")
                    # alternate the PSUM->SBUF drain between DVE and ACT
                    # (both idle in phase 3) so the final copies overlap
                    if c % 2 == 0:
                        nc.vector.tensor_copy(oc[:], acc_o[:])
                    else:
                        nc.scalar.activation(
                            oc[:], acc_o[:], mybir.ActivationFunctionType.Copy
                        )
                    nc.sync.dma_start(o[rs, hc], oc[:])

    _split_excess_waits(nc)
    return nc


_NC_CACHE = None


def _get_nc():
    global _NC_CACHE
    if _NC_CACHE is None:
        _NC_CACHE = _build_nc()
    return _NC_CACHE


def _is_causal_mask(mask: np.ndarray) -> bool:
    if mask.shape != (1, 1, S, S):
        return False
    m = mask[0, 0]
    tri = np.tril(np.ones((S, S), dtype=bool))
    return bool(np.all(m[tri] == 0.0) and np.all(m[~tri] <= _NEG_BIG))


def _reference_numpy(hidden_states, attention_mask, Wq, bq, Wk, bk, Wv, bv, Wo, bo):
    hs = hidden_states.astype(np.float64)
    out = np.empty((B, S, H), np.float64)
    for b in range(B):
        q = hs[b] @ Wq.T.astype(np.float64) + bq
        k = hs[b] @ Wk.T.astype(np.float64) + bk
        v = hs[b] @ Wv.T.astype(np.float64) + bv
        q = q.reshape(S, NH, HD).transpose(1, 0, 2)
        k = k.reshape(S, NH, HD).transpose(1, 0, 2)
        v = v.reshape(S, NH, HD).transpose(1, 0, 2)
        attn = np.einsum("nqd,nkd->nqk", q, k) / math.sqrt(HD)
        attn = attn + attention_mask[0].astype(np.float64)
        attn = attn - attn.max(axis=-1, keepdims=True)
        attn = np.exp(attn)
        attn = attn / attn.sum(axis=-1, keepdims=True)
        o = np.einsum("nqk,nkd->nqd", attn, v)
        o = o.transpose(1, 0, 2).reshape(S, H)
        out[b] = o @ Wo.T.astype(np.float64) + bo
    return out.astype(np.float32)


def _prepare_in_maps(hidden_states, Wq, bq, Wk, bk, Wv, bv, Wo):
    scale = 1.0 / math.sqrt(HD)
    bf = ml_dtypes.bfloat16
    masks = np.zeros((P, 4, 512), np.float32)
    kk = np.arange(P)[:, None]
    qq = np.arange(512)[None, :]
    for r in range(4):
        masks[:, r, :] = (qq >= kk + P * r).astype(np.float32)
    masks = masks.astype(bf).reshape(512, 512)

    shard_maps = []
    for r in range(4):
        ds = slice(DSH * r, DSH * (r + 1))
        wts = np.concatenate(
            [
                np.ascontiguousarray((Wq[ds, :] * scale).T).astype(bf),
                np.ascontiguousarray(Wk[ds, :].T).astype(bf),
                np.ascontiguousarray(Wv[ds, :].T).astype(bf),
                np.ascontiguousarray(Wo[:, ds].T).astype(bf).reshape(2048, 512),
                masks,
            ],
            axis=0,
        )
        bias = np.concatenate(
            [
                np.ascontiguousarray((bq[ds] * scale).reshape(HPC, HD).T).astype(
                    np.float32
                ),
                np.ascontiguousarray(bk[ds].reshape(HPC, HD).T).astype(np.float32),
                np.tile(bv[ds][None, :], (P, 1)).astype(np.float32),
            ],
            axis=1,
        )
        shard_maps.append((wts, bias))

    hts = [
        np.ascontiguousarray(hidden_states[b].T)
        .astype(bf)
        .reshape(8192, 512)
        for b in range(B)
    ]

    in_maps = []
    for c in range(NCORES):
        b, r = divmod(c, 4)
        wts, bias = shard_maps[r]
        wall = np.concatenate([hts[b], wts], axis=0)
        in_maps.append({"wall": wall, "bias": bias})
    return in_maps


def _assemble_output(partials, bo):
    out = np.zeros((B, S, H), np.float32)
    for c in range(NCORES):
        out[c // 4] += partials[c].astype(np.float32)
    out += bo[None, None, :]
    return out


def kernel(hidden_states, attention_mask, Wq, bq, Wk, bk, Wv, bv, Wo, bo):
    hidden_states = np.asarray(hidden_states, dtype=np.float32)
    attention_mask = np.asarray(attention_mask, dtype=np.float32)
    Wq, bq = np.asarray(Wq, np.float32), np.asarray(bq, np.float32)
    Wk, bk = np.asarray(Wk, np.float32), np.asarray(bk, np.float32)
    Wv, bv = np.asarray(Wv, np.float32), np.asarray(bv, np.float32)
    Wo, bo = np.asarray(Wo, np.float32), np.asarray(bo, np.float32)

    if not _is_causal_mask(attention_mask):
        # The device kernel exploits the causal structure; any other mask
        # falls back to an exact host computation.
        return _reference_numpy(
            hidden_states, attention_mask, Wq, bq, Wk, bk, Wv, bv, Wo, bo
        )

    in_maps = _prepare_in_maps(hidden_states, Wq, bq, Wk, bk, Wv, bv, Wo)
    nc = _get_nc()
    res = run_bass_kernel_spmd(nc, in_maps, core_ids=list(range(NCORES)))
    return _assemble_output([res.results[c]["o"] for c in range(NCORES)], bo)
